# revision 3
# baseline (speedup 1.0000x reference)
"""Deformable warp (bilinear grid_sample with shared displacement field) on 8 trn2 cores.

Problem: source [8,16,512,512] f32, displacement [1,2,512,512] f32 (shared over batch).
out[b,c,y,x] = bilinear_sample(source[b,c], x + dx[y,x]*255.5, y + dy[y,x]*255.5),
align_corners=True, zero padding.

Strategy (v5; ~405us vs 959us baseline):
  - fp16 "quad slab" built on host: qslab[i] = [ext[i] | ext[i+512] | ext[i+1] |
    ext[i+513]], each 128 fp16, so ONE contiguous 1KB run per output pixel holds
    all four bilinear corners in weight order (one DMA descriptor per pixel;
    half the HBM traffic of the f32 design).
  - Spatial sharding: core q computes output rows [64q, 64q+64) for ALL batches
    and channels. No cross-core communication.
  - Per core: DVE computes sampling coords/weights/indices from the displacement
    rows (preamble); 256 per-chunk indirect DMAs gather one 1KB run per output
    pixel (one index per partition per call - the only indirect-DMA shape the HW
    DGE supports; measured ~1.4us/call, Pool-engine serialized - the kernel's
    floor). The corner-weight multiply runs on the ACT engine (per-partition
    f32 scale, own SBUF ports, never blocks the Q7 SWDGE descriptor generator -
    DVE ops do block it, measured strictly additive). The two pairwise corner
    adds run on DVE in packed fp16; fp16 writeback in SBUF-native layout
    (contiguous 4KB/partition) which the host reassembles + casts to f32.

Zero-padding semantics: fetch base clamped into the slab, weights of
out-of-image corners zeroed (slab has generous zero padding).
"""

import sys

sys.path.insert(0, "/opt/trn_rl_repo")

import numpy as np

import concourse.bass as bass
import concourse.bacc as bacc
import concourse.mybir as mybir
import concourse.tile as tile

F32 = mybir.dt.float32
F16 = mybir.dt.float16
I32 = mybir.dt.int32

B, C, H, W = 8, 16, 512, 512
BC = B * C  # 128
NCORES = 8
ROWS = H // NCORES  # 64 output rows per core
NPX = ROWS * W  # 32768 pixels per core
CHUNKS = NPX // 128  # 256 chunks of 128 pixels
NCH = 16  # chunks per pipeline tile (2048 px)
NTILES = CHUNKS // NCH  # 16
SPLIT_CH = 0  # chunks of each tile whose weight-mult runs on GPSIMD (rest on DVE)
STAGES = "all"  # "all" | "gather" (debug: gather+writeback only, no compute)
MULT_ENG = "act"  # "act" | "dve": engine for the corner-weight multiply
BUFS = 2  # tile-pool double/quad buffering depth

# Quad-slab geometry: ext = [FRONT zero rows][H*W pixel rows][BACK zero rows],
# each row 128 fp16. qslab[i] = ext[i] | ext[i+512] | ext[i+1] | ext[i+513].
# Fetch base for a pixel: i = (y0m*512 + xb) + FRONT with y0m,xb in [-1, 511]:
# min i = FRONT - 513 >= 0 -> FRONT = 513; max needed ext row = i + 513 with
# max i = FRONT + 262143 -> BACK = 514 (same as baseline).
FRONT = 513
BACK = 514
NEXT = FRONT + H * W + BACK
NZ4 = NEXT - 513  # quad-slab rows (rows i where ext[i+513] exists)

AluOp = mybir.AluOpType


def _ap(handle, offset, dims):
    return bass.AP(handle, offset, [list(d) for d in dims])


def build_bass(reps=1, split=None, stages=None, scratch=None, mult_eng=None,
               bufs=2):
    global SPLIT_CH, STAGES, MULT_ENG, BUFS
    if split is not None:
        SPLIT_CH = split
    if stages is not None:
        STAGES = stages
    if mult_eng is not None:
        MULT_ENG = mult_eng
    BUFS = bufs
    kw = {"dynamic_dma_scratch_size": scratch} if scratch else {}
    nc = bacc.Bacc(**kw)
    nc.num_devices = NCORES

    qslab = nc.declare_dram_parameter("qslab", [NZ4, 4 * BC], F16, isOutput=False)
    disp = nc.declare_dram_parameter("disp", [2, ROWS, W], F32, isOutput=False)
    tabs = nc.declare_dram_parameter("tabs", [128, 2 * CHUNKS], F32, isOutput=False)
    out = nc.declare_dram_parameter("out", [128, CHUNKS, BC], F16, isOutput=True)

    with tile.TileContext(nc) as tc:
        with (
            tc.tile_pool(name="res", bufs=1) as res,
            tc.tile_pool(name="gat", bufs=BUFS) as gat,
            tc.tile_pool(name="ot", bufs=BUFS) as ot,
        ):
            v = nc.vector
            _tagn = [0]

            def rtile(shape, dtype):
                _tagn[0] += 1
                return res.tile(shape, dtype, tag=f"rt{_tagn[0]}", name=f"rt{_tagn[0]}")

            # ---- resident tensors -------------------------------------------------
            # pixel p (raster within this core's 64 rows) lives at
            # [partition = p % 128, chunk = p // 128]; chunk = 4*cy + cx where
            # y_local = cy, x = (p%128) + 128*cx.
            dxy = rtile([128, 2 * CHUNKS], F32)
            wts = rtile([128, CHUNKS, 4], F32)
            wts16 = rtile([128, CHUNKS, 4], F16)
            idx = rtile([128, CHUNKS], I32)

            # displacement load, both channels in one DMA:
            # value at (part, (ch*ROWS+cy)*4 + cx) = disp[ch, cy, part + 128*cx]
            nc.sync.dma_start(
                out=dxy[:],
                in_=_ap(disp, 0, [(1, 128), (W, 2 * ROWS), (128, 4)]),
            )
            dx = dxy[:, 0:CHUNKS]
            dy = dxy[:, CHUNKS:2 * CHUNKS]

            # per-pixel normalized-coordinate tables (host-arranged, bit-exact
            # jnp.linspace values): xs_pix | ys_pix halves
            tabt = rtile([128, 2 * CHUNKS], F32)
            nc.sync.dma_start(out=tabt[:], in_=tabs[:])
            xs_pix = tabt[:, 0:CHUNKS]
            ys_pix = tabt[:, CHUNKS:2 * CHUNKS]

            # sampling coords in pixel space, matching the reference op-for-op:
            #   g = table + d;  pix = (g + 1) * 0.5 * (size-1)
            gx = rtile([128, CHUNKS], F32)
            gy = rtile([128, CHUNKS], F32)
            v.tensor_tensor(out=gx[:], in0=xs_pix, in1=dx, op=AluOp.add)
            v.tensor_scalar(out=gx[:], in0=gx[:], scalar1=1.0, scalar2=(W - 1) / 2.0,
                            op0=AluOp.add, op1=AluOp.mult)
            v.tensor_tensor(out=gy[:], in0=ys_pix, in1=dy, op=AluOp.add)
            v.tensor_scalar(out=gy[:], in0=gy[:], scalar1=1.0, scalar2=(H - 1) / 2.0,
                            op0=AluOp.add, op1=AluOp.mult)

            def floor_frac(g, lim):
                """returns (frac, w0=1-frac, v0, v1, gb=clamp(g0,-1,lim-1))"""
                t_i = rtile([128, CHUNKS], I32)
                v.tensor_copy(out=t_i[:], in_=g[:])
                tf = rtile([128, CHUNKS], F32)
                v.tensor_copy(out=tf[:], in_=t_i[:])
                adj = rtile([128, CHUNKS], F32)
                v.tensor_tensor(out=adj[:], in0=tf[:], in1=g[:], op=AluOp.is_gt)
                g0 = rtile([128, CHUNKS], F32)
                v.tensor_tensor(out=g0[:], in0=tf[:], in1=adj[:], op=AluOp.subtract)
                fr = rtile([128, CHUNKS], F32)
                v.tensor_tensor(out=fr[:], in0=g[:], in1=g0[:], op=AluOp.subtract)
                w0 = rtile([128, CHUNKS], F32)
                v.tensor_scalar(out=w0[:], in0=fr[:], scalar1=-1.0, scalar2=1.0,
                                op0=AluOp.mult, op1=AluOp.add)
                m0 = rtile([128, CHUNKS], F32)
                m1 = rtile([128, CHUNKS], F32)
                v0 = rtile([128, CHUNKS], F32)
                v1 = rtile([128, CHUNKS], F32)
                v.tensor_scalar(out=m0[:], in0=g0[:], scalar1=0.0, scalar2=None, op0=AluOp.is_ge)
                v.tensor_scalar(out=m1[:], in0=g0[:], scalar1=float(lim - 1), scalar2=None, op0=AluOp.is_le)
                v.tensor_tensor(out=v0[:], in0=m0[:], in1=m1[:], op=AluOp.mult)
                v.tensor_scalar(out=m0[:], in0=g0[:], scalar1=-1.0, scalar2=None, op0=AluOp.is_ge)
                v.tensor_scalar(out=m1[:], in0=g0[:], scalar1=float(lim - 2), scalar2=None, op0=AluOp.is_le)
                v.tensor_tensor(out=v1[:], in0=m0[:], in1=m1[:], op=AluOp.mult)
                gb = rtile([128, CHUNKS], F32)
                v.tensor_scalar(out=gb[:], in0=g0[:], scalar1=-1.0, scalar2=float(lim - 1),
                                op0=AluOp.max, op1=AluOp.min)
                return fr, w0, v0, v1, gb

            fx, wx0, vx0, vx1, xb = floor_frac(gx, W)
            fy, wy0, vy0, vy1, yb = floor_frac(gy, H)

            # gather index first (unblocks the gather pipeline):
            # (yb*512 + xb) + FRONT, all values exact in f32
            idf = rtile([128, CHUNKS], F32)
            v.scalar_tensor_tensor(out=idf[:], in0=yb[:], scalar=float(W), in1=xb[:],
                                   op0=AluOp.mult, op1=AluOp.add)
            v.tensor_scalar(out=idf[:], in0=idf[:], scalar1=float(FRONT), scalar2=None,
                            op0=AluOp.add)
            v.tensor_copy(out=idx[:], in_=idf[:])

            # masked 1-D weights
            wxa = rtile([128, CHUNKS], F32)
            wxb = rtile([128, CHUNKS], F32)
            wya = rtile([128, CHUNKS], F32)
            wyb = rtile([128, CHUNKS], F32)
            v.tensor_tensor(out=wxa[:], in0=wx0[:], in1=vx0[:], op=AluOp.mult)
            v.tensor_tensor(out=wxb[:], in0=fx[:], in1=vx1[:], op=AluOp.mult)
            v.tensor_tensor(out=wya[:], in0=wy0[:], in1=vy0[:], op=AluOp.mult)
            v.tensor_tensor(out=wyb[:], in0=fy[:], in1=vy1[:], op=AluOp.mult)

            # corner weights, gathered-run order (y0x0, y1x0, y0x1, y1x1):
            for k, (a, b) in enumerate(((wya, wxa), (wyb, wxa), (wya, wxb), (wyb, wxb))):
                wk = _ap(wts.tensor, wts[:].offset + k, [(wts[:].ap[0][0], 128), (4, CHUNKS)])
                v.tensor_tensor(out=wk, in0=a[:], in1=b[:], op=AluOp.mult)
            v.tensor_copy(out=wts16[:], in_=wts[:])

            # ---- main pipeline ----------------------------------------------------
            import contextlib
            loop_ctx = tc.For_i(0, reps) if reps > 1 else contextlib.nullcontext()
            with loop_ctx:
                main_pipeline(nc, tc, v, qslab, out, wts, wts16, idx, gat, ot)

    return nc


def main_pipeline(nc, tc, v, qslab, out, wts, wts16, idx, gat, ot):
    for t in range(NTILES):
        # gathered tile: [part][chunk][4 corners x 128bc] fp16; per-chunk
        # indirect DMAs (one index per partition per call — the only shape
        # the HW DGE supports; each index -> one contiguous 1KB descriptor)
        g = gat.tile([128, NCH, 4 * BC], F16)
        gp = g[:].ap[0][0]
        ngather = 1 if STAGES == "compute" else NCH
        for c in range(ngather):
            cg = t * NCH + c
            nc.gpsimd.indirect_dma_start(
                out=g[:, c, :],
                out_offset=None,
                in_=qslab[:],
                in_offset=bass.IndirectOffsetOnAxis(ap=idx[:, cg:cg + 1], axis=0),
            )

        if STAGES == "gather":
            out_t = _ap(out, t * NCH * BC,
                        [(CHUNKS * BC, 128), (1, NCH * BC)])
            nc.sync.dma_start(out=out_t, in_=g[:, :, 0:BC])
            continue

        # multiply by corner weights. On "act": per-(chunk,corner) activations
        # with a per-partition scale — the ACT engine has its own SBUF ports
        # and never blocks the Q7 SWDGE descriptor generator (DVE ops do).
        wp = wts16[:].ap[0][0]
        woff = wts16[:].offset + t * NCH * 4
        if MULT_ENG == "act":
            Copy = mybir.ActivationFunctionType.Copy
            for c in range(NCH):
                cg = t * NCH + c
                for k in range(4):
                    sl = _ap(g.tensor, g[:].offset + c * 4 * BC + k * BC,
                             [(gp, 128), (1, BC)])
                    nc.scalar.activation(
                        out=sl, in_=sl, func=Copy,
                        scale=wts[:, cg, k:k + 1],
                    )
        else:
            g_m1 = _ap(g.tensor, g[:].offset,
                       [(gp, 128), (4 * BC, NCH), (BC, 4), (1, BC)])
            w_m1 = _ap(wts16.tensor, woff,
                       [(wp, 128), (4, NCH), (1, 4), (0, BC)])
            v.tensor_tensor(out=g_m1, in0=g_m1, in1=w_m1, op=AluOp.mult)

        # reduce the 4 corners: pairwise adds, all packed fp16
        h = ot.tile([128, NCH, 2 * BC], F16, tag="h", name=f"h_{t}")
        ga = _ap(g.tensor, g[:].offset,
                 [(gp, 128), (4 * BC, NCH), (1, 2 * BC)])
        gb2 = _ap(g.tensor, g[:].offset + 2 * BC,
                  [(gp, 128), (4 * BC, NCH), (1, 2 * BC)])
        v.tensor_tensor(out=h[:], in0=ga, in1=gb2, op=AluOp.add)
        o = ot.tile([128, NCH, BC], F16)
        hp = h[:].ap[0][0]
        ha = _ap(h.tensor, h[:].offset,
                 [(hp, 128), (2 * BC, NCH), (1, BC)])
        hb = _ap(h.tensor, h[:].offset + BC,
                 [(hp, 128), (2 * BC, NCH), (1, BC)])
        v.tensor_tensor(out=o[:], in0=ha, in1=hb, op=AluOp.add)

        # writeback: SBUF-native layout out[part, chunk, bc], contiguous
        # NCH*BC fp16 (4KB) per partition per tile
        out_t = _ap(out, t * NCH * BC,
                    [(CHUNKS * BC, 128), (1, NCH * BC)])
        nc.sync.dma_start(out=out_t, in_=o[:])


def linspace_tables():
    """The reference's jnp.linspace(-1, 1, size) values, bit-exact (computed on CPU)."""
    import jax

    with jax.default_device(jax.devices("cpu")[0]):
        xs = np.asarray(jax.numpy.linspace(-1.0, 1.0, W, dtype=np.float32))
        ys = np.asarray(jax.numpy.linspace(-1.0, 1.0, H, dtype=np.float32))
    return xs, ys


def coord_tables(q):
    """Per-pixel linspace tables in the kernel's [part, chunk] pixel layout, core q.
    Returns one [128, 2*CHUNKS] array: xs half | ys half."""
    xs, ys = linspace_tables()
    cx = np.arange(CHUNKS) % 4
    cy = np.arange(CHUNKS) // 4
    part = np.arange(128)
    xs_pix = xs[part[:, None] + 128 * cx[None, :]]
    ys_pix = np.broadcast_to(ys[q * ROWS + cy][None, :], (128, CHUNKS))
    return np.ascontiguousarray(np.hstack([xs_pix, ys_pix]), np.float32)


def build_qslab(source):
    """Channel-last fp16 quad slab: qslab[i] = ext[i]|ext[i+512]|ext[i+1]|ext[i+513]."""
    ext = np.zeros((NEXT, BC), np.float16)
    ext[FRONT:FRONT + H * W] = (
        source.transpose(2, 3, 0, 1).reshape(H * W, BC).astype(np.float16)
    )
    z = np.empty((NZ4, 4 * BC), np.float16)
    z[:, 0 * BC:1 * BC] = ext[0:NZ4]
    z[:, 1 * BC:2 * BC] = ext[512:512 + NZ4]
    z[:, 2 * BC:3 * BC] = ext[1:1 + NZ4]
    z[:, 3 * BC:4 * BC] = ext[513:513 + NZ4]
    return z


def make_in_maps(source, displacement):
    source = np.ascontiguousarray(source, dtype=np.float32)
    displacement = np.ascontiguousarray(displacement, dtype=np.float32)
    assert source.shape == (B, C, H, W)
    assert displacement.shape == (1, 2, H, W)
    z = build_qslab(source)
    d = displacement[0]
    in_maps = []
    for q in range(NCORES):
        in_maps.append({
            "qslab": z,
            "disp": np.ascontiguousarray(d[:, q * ROWS:(q + 1) * ROWS, :]),
            "tabs": coord_tables(q),
        })
    return in_maps


_NC_CACHE = None


def _get_nc():
    global _NC_CACHE
    if _NC_CACHE is None:
        _NC_CACHE = build_bass()
        if not _NC_CACHE.is_finalized():
            _NC_CACHE.finalize()
    return _NC_CACHE


def assemble_output(outs):
    # out_np [128, CHUNKS, 128] fp16: [part, (cy, cx), bc] with
    # x = cx*128 + part, y_local = cy, bc = b*C + c
    planes = []
    for o in outs:
        a = o.reshape(128, ROWS, 4, BC).transpose(3, 1, 2, 0)  # [bc, cy, cx, part]
        planes.append(a.reshape(BC, ROWS, W))
    full = np.concatenate(planes, axis=1)  # [bc, H, W]
    return np.ascontiguousarray(full.reshape(B, C, H, W).astype(np.float32))


def kernel(source, displacement):
    from concourse.bass_utils import run_bass_kernel_spmd

    in_maps = make_in_maps(source, displacement)
    res = run_bass_kernel_spmd(_get_nc(), in_maps, list(range(NCORES)))
    return assemble_output([res.results[q]["out"] for q in range(NCORES)])


def _pjrt_callable(nc, in_maps):
    """Build a jitted sharded callable for `nc` with device-resident inputs.

    Returns run() -> wall seconds for one execution (inputs stay resident;
    fresh zero output buffers are donated each call, excluded from the timed
    region)."""
    import time

    import jax
    import concourse.mybir as mybir
    from jax.sharding import Mesh, PartitionSpec, NamedSharding
    from jax.experimental.shard_map import shard_map
    from concourse.bass2jax import (
        _bass_exec_p, partition_id_tensor, install_neuronx_cc_hook)

    install_neuronx_cc_hook()
    n_cores = NCORES
    in_names, out_names, out_avals, zero_outs = [], [], [], []
    partition_name = nc.partition_id_tensor.name if nc.partition_id_tensor else None
    for alloc in nc.m.functions[0].allocations:
        if not isinstance(alloc, mybir.MemoryLocationSet):
            continue
        name = alloc.memorylocations[0].name
        if alloc.kind == "ExternalInput":
            if name != partition_name:
                in_names.append(name)
        elif alloc.kind == "ExternalOutput":
            out_names.append(name)
            shape = tuple(alloc.tensor_shape)
            dtype = mybir.dt.np(alloc.dtype)
            out_avals.append(jax.core.ShapedArray(shape, dtype))
            zero_outs.append(np.zeros(shape, dtype))
    n_params = len(in_names)
    all_in_names = in_names + out_names
    if partition_name is not None:
        all_in_names.append(partition_name)
    donate = tuple(range(n_params, n_params + len(out_avals)))

    def _body(*args):
        operands = list(args)
        if partition_name is not None:
            operands.append(partition_id_tensor())
        outs = _bass_exec_p.bind(
            *operands, out_avals=tuple(out_avals), in_names=tuple(all_in_names),
            out_names=tuple(out_names), lowering_input_output_aliases=(),
            sim_require_finite=True, sim_require_nnan=True, nc=nc)
        return tuple(outs)

    devices = jax.devices()[:n_cores]
    mesh = Mesh(np.asarray(devices), ("core",))
    in_specs = (PartitionSpec("core"),) * (n_params + len(out_avals))
    out_specs = (PartitionSpec("core"),) * len(out_names)
    sharded = jax.jit(
        shard_map(_body, mesh=mesh, in_specs=in_specs, out_specs=out_specs,
                  check_rep=False),
        donate_argnums=donate, keep_unused=True)
    sh = NamedSharding(mesh, PartitionSpec("core"))
    concat_in = [
        jax.device_put(
            np.concatenate([np.asarray(in_maps[c][nm]) for c in range(n_cores)],
                           axis=0), sh)
        for nm in in_names
    ]
    for x in concat_in:
        x.block_until_ready()

    def run():
        zs = [jax.device_put(
            np.zeros((n_cores * z.shape[0], *z.shape[1:]), z.dtype), sh)
            for z in zero_outs]
        for z in zs:
            z.block_until_ready()
        t0 = time.time()
        outs = sharded(*concat_in, *zs)
        for o in outs:
            o.block_until_ready()
        return time.time() - t0

    return run


def measure_hw(source, displacement, reps=4097, warm=3):
    """Per-invocation HW time: slope between device-looped programs
    (reps vs 1) with device-resident inputs, min over `warm` samples."""
    in_maps = make_in_maps(source, displacement)

    ncR = build_bass(reps=reps)
    ncR.finalize()
    nc1 = _get_nc()

    runR = _pjrt_callable(ncR, in_maps)
    run1 = _pjrt_callable(nc1, in_maps)
    runR(); run1()  # warm compile
    tRs, t1s = [], []
    for _ in range(warm):
        tRs.append(runR())
        t1s.append(run1())
    tR, t1 = min(tRs), min(t1s)
    t_ns = (tR - t1) / (reps - 1) * 1e9
    return t_ns, {"wall_reps": tR, "wall_1": t1, "reps": reps}


if __name__ == "__main__":
    nc = build_bass()
    print("built ok:", len(list(nc.all_instructions())), "instructions")
